# revision 33
# baseline (speedup 1.0000x reference)
"""Biaffine attention kernel for Trainium2, data-parallel over 8 NeuronCores.

Math (per batch b):
    xp = Wf @ x[b] + bf          (128, L)
    yp = Wa @ y[b] + ba          (128, L)
    scores = xp @ yp.T           (128, 128)   contraction over L
    attn = softmax(scores, -1) / sqrt(L)
    out[b] = attn @ (xp + yp)    (128, L)

Distribution: batch dim (32) sharded 4-per-core across 8 cores; weights
replicated. No collectives.

Key structure (all fp16 on the wire, fp32 PSUM accumulation):
  - x/y staged to HBM as fp16 (host cast): halves input DMA traffic.
  - Transposed activations xpT/ypT computed DIRECTLY on TensorE with the
    input chunk as the stationary operand (out = x_c.T @ WfT), skipping
    the separate natural-projection + transpose passes entirely.
  - Bias added during PSUM evacuation as a broadcast row tensor
    (scalar_tensor_tensor on DVE / GpSimd), so scores need no fixups.
  - scores accumulate over 64 chunk matmuls into one PSUM bank.
  - Softmax rowwise: DVE max-reduce, ACT exp with fused -max bias and
    sum accumulation, DVE reciprocal; 1/sqrt(L) folded in.
  - Output fused: out = attn@(xp+yp) = (attn@Wf)@x + (attn@Wa)@y
    + (attn@(bf+ba)) broadcast. AfT/AaT are tiny 128x128 matmuls from
    attnT; z is never materialized; the bias term rides the ACT
    evacuation as a per-partition bias. Output stored fp16, host upcast.
"""

import numpy as np

P = 128
L = 8192
B = 32
NCORES = 8
BPC = B // NCORES  # batches per core
SQRT_L = float(np.sqrt(float(L)))

IN_TILE = 2048  # HBM->SBUF dma tile (4 KiB/partition fp16)
GRP = 512  # pass-B PSUM group (4 x 128-col chunks per bank)
OUT_TILE = 2048  # SBUF->HBM out staging tile


def _patch_tail_drain(tile, mybir, ScopedClock):
    """This container's walrus rejects >1 sync wait on the kernel-tail Drain
    (setupSyncWait: 'Too many sync wait commands'). Spread the tail-drain
    waits across a chain of drains, one wait each."""
    if getattr(tile.TileContext, "_drain_split_patched", False):
        return

    def _split_drain_and_barrier(self, tick_clock, wait_clock):
        nc = self.nc
        drain_inst = nc.sync.drain()
        wait_clock.add_sem_waits(
            drain_inst.ins, ScopedClock({None: tick_clock.global_clock})
        )
        si = drain_inst.ins.sync_info
        if si is not None and si.on_wait is not None and len(si.on_wait) > 1:
            waits = list(si.on_wait)
            si.on_wait = waits[:1]
            for w in waits[1:]:
                extra = nc.sync.drain()
                esi = extra.ins.sync_info
                if esi is None:
                    extra.ins.sync_info = mybir.SyncInfo(on_wait=[w], on_update=[])
                else:
                    ow = list(esi.on_wait) if esi.on_wait else []
                    ow.append(w)
                    esi.on_wait = ow
        nc.all_engine_barrier()
        assert self.sems is not None
        popped = nc._tile_sem_poison_stack.pop()
        assert popped is self._sem_poison
        nc.clear_and_free_semaphores(list(self.sems.allocated().values()))
        nc.all_engine_barrier()

    tile.TileContext._drain_and_barrier = _split_drain_and_barrier
    tile.TileContext._drain_split_patched = True


def _split_excess_waits(nc, mybir, max_waits=1):
    """Walrus in this container rejects instructions carrying more than a
    couple of sync waits ('Too many sync wait commands'). Hoist excess waits
    onto dedicated same-engine NoOps inserted just before the instruction."""
    ctr = 0
    for blk in nc.m.functions[0].blocks:
        new_insts = []
        for inst in blk.instructions:
            si = inst.sync_info
            if si is not None and si.on_wait and len(si.on_wait) > max_waits:
                waits = list(si.on_wait)
                excess, keep = waits[:-max_waits], waits[-max_waits:]
                si.on_wait = keep
                for i in range(0, len(excess), max_waits):
                    ctr += 1
                    nop = mybir.InstNoOp(
                        name=f"I-waitsplit-{ctr}",
                        sync_info=mybir.SyncInfo(
                            on_wait=excess[i : i + max_waits], on_update=[]
                        ),
                        bass_nofuse=True,
                        engine=inst.engine,
                    )
                    nc.register_instruction(nop)
                    new_insts.append(nop)
            new_insts.append(inst)
        blk.instructions = new_insts


def build_nc(bpc=BPC, seq=L):
    import concourse.bass as bass
    import concourse.mybir as mybir
    import concourse.tile as tile
    from concourse.masks import make_identity
    from concourse.vector_clock import ScopedClock

    _patch_tail_drain(tile, mybir, ScopedClock)

    f32 = mybir.dt.float32
    f16 = mybir.dt.float16
    AF = mybir.ActivationFunctionType
    ALU = mybir.AluOpType
    AX = mybir.AxisListType

    sqrt_l = float(np.sqrt(float(seq)))
    nin = seq // IN_TILE  # dma tiles per batch tensor
    ngrp = seq // GRP  # pass-B psum groups per batch
    gpc = IN_TILE // GRP  # groups per dma tile
    ntr = seq // P  # 128-col chunks per batch

    nc = bass.Bass("TRN2", target_bir_lowering=False, debug=False)
    x_d = nc.dram_tensor("x", [bpc, P, seq], f16, kind="ExternalInput").ap()
    y_d = nc.dram_tensor("y", [bpc, P, seq], f16, kind="ExternalInput").ap()
    wf_d = nc.dram_tensor("wf", [P, P], f16, kind="ExternalInput").ap()
    wa_d = nc.dram_tensor("wa", [P, P], f16, kind="ExternalInput").ap()
    wft_d = nc.dram_tensor("wft", [P, P], f16, kind="ExternalInput").ap()
    wat_d = nc.dram_tensor("wat", [P, P], f16, kind="ExternalInput").ap()
    bfb_d = nc.dram_tensor("bfb", [P, GRP], f16, kind="ExternalInput").ap()
    bzb_d = nc.dram_tensor("bzb", [P, P], f16, kind="ExternalInput").ap()
    lbf_d = nc.dram_tensor("lbf", [1, P], f16, kind="ExternalInput").ap()
    bar_d = nc.dram_tensor("bar", [1, P], f16, kind="ExternalInput").ap()
    out_d = nc.dram_tensor("out", [bpc, P, seq], f16, kind="ExternalOutput").ap()

    with tile.TileContext(nc) as tc:
        with (
            tc.tile_pool(name="consts", bufs=1) as consts,
            tc.tile_pool(name="xin", bufs=3) as xin_pool,
            tc.tile_pool(name="acts", bufs=2) as acts_pool,
            tc.tile_pool(name="sm", bufs=2) as sm_pool,
            tc.tile_pool(name="outs", bufs=2) as out_pool,
            tc.tile_pool(name="pxt", bufs=2, space="PSUM") as psum_xt,
            tc.tile_pool(name="pyt", bufs=2, space="PSUM") as psum_yt,
            tc.tile_pool(name="psc", bufs=1, space="PSUM") as psum_sc,
            tc.tile_pool(name="pout", bufs=2, space="PSUM") as psum_out,
            tc.tile_pool(name="psm", bufs=1, space="PSUM") as psum_sm,
        ):
            # ---- constants (scalar ring: idle at start, ahead of stores) ----
            wf16 = consts.tile([P, P], f16)
            nc.scalar.dma_start(wf16, wf_d)
            wa16 = consts.tile([P, P], f16)
            nc.scalar.dma_start(wa16, wa_d)
            wfT = consts.tile([P, P], f16)
            nc.scalar.dma_start(wfT, wft_d)
            waT = consts.tile([P, P], f16)
            nc.scalar.dma_start(waT, wat_d)
            bfb = consts.tile([P, GRP], f16)
            nc.scalar.dma_start(bfb, bfb_d)
            bzb = consts.tile([P, P], f16)
            nc.scalar.dma_start(bzb, bzb_d)
            lbf = consts.tile([1, P], f16)
            nc.scalar.dma_start(lbf, lbf_d)
            bar = consts.tile([1, P], f16)
            nc.scalar.dma_start(bar, bar_d)
            ids = consts.tile([P, P], f16)
            make_identity(nc, ids)

            # Pre-warm the PE HAM clock gate during the initial input DMA
            # wait: ~24 dummy matmuls (~2.6us cold) flip the 4096-cycle
            # activity window to K=8/8 before the real stream begins.
            pwarm = psum_xt.tile([P, GRP], f32, tag="px", name="pwarm")
            for w in range(24):
                nc.tensor.matmul(
                    pwarm[:, (w % 4) * P : (w % 4 + 1) * P],
                    wfT,
                    wfT,
                    start=True,
                    stop=True,
                )

            # ---- input loads (x: SP ring, y: GpSimd ring) ----
            # Only batches 0-1 up front: 32 concurrent transfers at startup
            # starve the first-needed tile of bandwidth (~9 us extra PE
            # stall measured); later batches stream in from the loop.
            tiles = {}

            def emit_load(b):
                for h in range(nin):
                    x_t = xin_pool.tile([P, IN_TILE], f16, tag=f"x{h}", name=f"x{h}")
                    y_t = xin_pool.tile([P, IN_TILE], f16, tag=f"y{h}", name=f"y{h}")
                    hs = slice(h * IN_TILE, (h + 1) * IN_TILE)
                    nc.sync.dma_start(x_t, x_d[b, :, hs])
                    nc.gpsimd.dma_start(y_t, y_d[b, :, hs])
                    tiles[(b, h)] = (x_t, y_t)

            emit_load(0)
            emit_load(1)

            st = {}  # per-batch live tiles

            def emit_passB(b, glo, ghi):
                """Projection groups [glo, ghi): PE matmuls, DVE xpT evac
                with bf broadcast-add, ACT ypT evac plain (y bias lands on
                scores as a rank-1 term). xsum partials ride the tile
                boundaries."""
                s = st[b]
                for g in range(glo, ghi):
                    ht = g // gpc
                    px = psum_xt.tile([P, GRP], f32, tag="px", name="px")
                    py = psum_yt.tile([P, GRP], f32, tag="py", name="py")
                    for t in range(4):
                        lo = (g % gpc) * GRP + t * P
                        cs = slice(lo, lo + P)
                        ts_ = slice(t * P, (t + 1) * P)
                        nc.tensor.matmul(
                            px[:, ts_],
                            tiles[(b, ht)][0][:, cs],
                            wfT,
                            start=True,
                            stop=True,
                        )
                    for t in range(4):
                        lo = (g % gpc) * GRP + t * P
                        cs = slice(lo, lo + P)
                        ts_ = slice(t * P, (t + 1) * P)
                        nc.tensor.matmul(
                            py[:, ts_],
                            tiles[(b, ht)][1][:, cs],
                            waT,
                            start=True,
                            stop=True,
                        )
                    gs = slice(g * GRP, (g + 1) * GRP)
                    nc.vector.scalar_tensor_tensor(
                        out=s["xpT"][:, gs],
                        in0=px,
                        scalar=1.0,
                        in1=bfb,
                        op0=ALU.mult,
                        op1=ALU.add,
                    )
                    nc.scalar.activation(
                        out=s["ypT"][:, gs], in_=py, func=AF.Identity, bias=0.0
                    )
                    if g % gpc == gpc - 1:
                        # last group of dma tile ht: fold its xsum partial,
                        # alternating DVE reduce / ACT accum_out so neither
                        # engine eats the whole ~7us/batch reduction.
                        ht_ = g // gpc
                        if ht_ % 2 == 0:
                            nc.vector.tensor_reduce(
                                out=s["xsums"][:, ht_ : ht_ + 1],
                                in_=tiles[(b, ht_)][0],
                                axis=AX.X,
                                op=ALU.add,
                            )
                        else:
                            xjunk = sm_pool.tile(
                                [P, IN_TILE], f16, tag="xjunk", name="xjunk"
                            )
                            nc.scalar.activation(
                                out=xjunk,
                                in_=tiles[(b, ht_)][0],
                                func=AF.Identity,
                                bias=0.0,
                                accum_out=s["xsums"][:, ht_ : ht_ + 1],
                            )

            def emit_prep(b):
                """xsum -> xpsum row (for the y-bias rank-1 on scores)."""
                s = st[b]
                xsf = sm_pool.tile([P, 1], f32, tag="xsf", name="xsf")
                nc.vector.tensor_reduce(
                    out=xsf, in_=s["xsums"], axis=AX.X, op=ALU.add
                )
                xsc = sm_pool.tile([P, 1], f16, tag="xsc", name="xsc")
                nc.vector.tensor_copy(out=xsc, in_=xsf)
                psct = s["psct"]
                # pxp = xsum.T @ WfT -> [1, 128] row = (Wf @ xsum).T
                nc.tensor.matmul(
                    psct[0:1, P : 2 * P], xsc, wfT, start=True, stop=True
                )
                # xpsum_row = pxp + L*bf
                nc.vector.scalar_tensor_tensor(
                    out=s["xpsum"],
                    in0=psct[0:1, P : 2 * P],
                    scalar=1.0,
                    in1=lbf,
                    op0=ALU.mult,
                    op1=ALU.add,
                )

            def emit_scores(b):
                s = st[b]
                ps = s["psct"][:, 0:P]
                for c in range(ntr):
                    cs = slice(c * P, (c + 1) * P)
                    nc.tensor.matmul(
                        ps,
                        s["xpT"][:, cs],
                        s["ypT"][:, cs],
                        start=(c == 0),
                        stop=False,
                    )
                # + xpsum (x) ba : the y-side bias term, K=1 rank-1 matmul
                nc.tensor.matmul(ps, s["xpsum"], bar, start=False, stop=True)

            def emit_softmax_head(b):
                s = st[b]
                ps = s["psct"][:, 0:P]
                negmx = sm_pool.tile([P, 1], f32, tag="negmx", name="negmx")
                nc.vector.tensor_reduce(
                    out=negmx, in_=ps, axis=AX.X, op=ALU.max, negate=True
                )
                e = sm_pool.tile([P, P], f32, tag="e", name="e")
                se = sm_pool.tile([P, 1], f32, tag="se", name="se")
                nc.scalar.activation(
                    out=e, in_=ps, func=AF.Exp, bias=negmx, scale=1.0, accum_out=se
                )
                s["e"], s["se"] = e, se

            def emit_softmax_tail(b):
                """SBUF-only chain: runs on GpSimd (DVE stays on evacs).
                reciprocal only exists on DVE."""
                s = st[b]
                sse = sm_pool.tile([P, 1], f32, tag="sse", name="sse")
                nc.gpsimd.tensor_scalar_mul(sse, s["se"], sqrt_l)
                rcp = sm_pool.tile([P, 1], f32, tag="rcp", name="rcp")
                nc.vector.reciprocal(rcp, sse)
                attn = sm_pool.tile([P, P], f16, tag="attn", name="attn")
                nc.gpsimd.tensor_scalar_mul(attn, s["e"], rcp)
                # ab = attn @ (bf+ba): free-dim weighted row-sum (DVE;
                # GpSimd doesn't lower scalar_tensor_tensor)
                junk = sm_pool.tile([P, P], f16, tag="junk", name="junk")
                ab = sm_pool.tile([P, 1], f32, tag="ab", name="ab")
                nc.vector.scalar_tensor_tensor(
                    out=junk,
                    in0=attn,
                    scalar=1.0,
                    in1=bzb,
                    op0=ALU.mult,
                    op1=ALU.mult,
                    accum_out=ab,
                )
                s["attn"], s["ab"] = attn, ab

            def emit_smalls_pe(b):
                """attnT transpose + AfT/AaT tiny matmuls (PE/DVE chain)."""
                s = st[b]
                pat = psum_sm.tile([P, P], f16, tag="pat", name="pat")
                nc.tensor.transpose(pat, s["attn"], ids)
                attnT = sm_pool.tile([P, P], f16, tag="attnT", name="attnT")
                nc.vector.tensor_copy(out=attnT, in_=pat)
                pwt = psum_out.tile([P, GRP], f32, tag="po", name="pwt")
                nc.tensor.matmul(pwt[:, 0:P], wf16, attnT, start=True, stop=True)
                AfT = sm_pool.tile([P, P], f16, tag="AfT", name="AfT")
                nc.vector.tensor_copy(out=AfT, in_=pwt[:, 0:P])
                nc.tensor.matmul(
                    pwt[:, P : 2 * P], wa16, attnT, start=True, stop=True
                )
                AaT = sm_pool.tile([P, P], f16, tag="AaT", name="AaT")
                nc.vector.tensor_copy(out=AaT, in_=pwt[:, P : 2 * P])
                s["AfT"], s["AaT"] = AfT, AaT

            def emit_out(b):
                s = st[b]
                nout = seq // OUT_TILE
                cpo = OUT_TILE // GRP
                idx = 0
                for h in range(nout):
                    ot = out_pool.tile([P, OUT_TILE], f16, tag="ot", name="ot")
                    for cc in range(cpo):
                        c0 = h * OUT_TILE + cc * GRP
                        ht = c0 // IN_TILE
                        lo = c0 % IN_TILE
                        cs = slice(lo, lo + GRP)
                        po = psum_out.tile([P, GRP], f32, tag="po", name="po")
                        nc.tensor.matmul(
                            po, s["AfT"], tiles[(b, ht)][0][:, cs],
                            start=True, stop=False,
                        )
                        nc.tensor.matmul(
                            po, s["AaT"], tiles[(b, ht)][1][:, cs],
                            start=False, stop=True,
                        )
                        ots = ot[:, cc * GRP : (cc + 1) * GRP]
                        # split evacuation ACT/DVE to balance engine load
                        if idx % 2 == 1:
                            nc.vector.tensor_scalar_add(ots, po, s["ab"])
                        else:
                            nc.scalar.activation(
                                out=ots, in_=po, func=AF.Identity, bias=s["ab"]
                            )
                        idx += 1
                    hs = slice(h * OUT_TILE, (h + 1) * OUT_TILE)
                    # SP ring: only 4 x-loads/batch ride it, ACT ring is busy
                    # with evacuations
                    nc.sync.dma_start(out_d[b, :, hs], ot)

            # ---- software-pipelined batch loop ----
            for b in range(bpc):
                if b + 2 < bpc:
                    emit_load(b + 2)
                st[b] = {
                    "xpT": acts_pool.tile([P, seq], f16, tag="xpT", name="xpT"),
                    "ypT": acts_pool.tile([P, seq], f16, tag="ypT", name="ypT"),
                    "xsums": sm_pool.tile([P, nin], f32, tag="xsums", name="xsums"),
                    "psct": psum_sc.tile([P, 2 * P], f32, tag="ps", name="ps"),
                    "xpsum": sm_pool.tile([1, P], f16, tag="xpsum", name="xpsum"),
                }
                if b > 0:
                    emit_softmax_tail(b - 1)
                emit_passB(b, 0, ngrp // 2)
                if b > 0:
                    emit_smalls_pe(b - 1)
                emit_passB(b, ngrp // 2, ngrp)
                emit_prep(b)
                emit_scores(b)
                emit_softmax_head(b)
                if b > 0:
                    emit_out(b - 1)
                    del st[b - 1]
            emit_softmax_tail(bpc - 1)
            emit_smalls_pe(bpc - 1)
            emit_out(bpc - 1)

    _split_excess_waits(nc, mybir, max_waits=1)
    return nc


_nc_cache = {}


def _get_nc():
    key = (BPC, L)
    if key not in _nc_cache:
        _nc_cache[key] = build_nc(BPC, L)
    return _nc_cache[key]


def make_in_maps(x, y, Wf, bf, Wa, ba):
    """Host staging: fp16 casts + layout-only prep, sharded per core."""
    x16 = np.asarray(x, dtype=np.float16)
    y16 = np.asarray(y, dtype=np.float16)
    Wf = np.asarray(Wf, dtype=np.float32)
    bf = np.asarray(bf, dtype=np.float32)
    Wa = np.asarray(Wa, dtype=np.float32)
    ba = np.asarray(ba, dtype=np.float32)

    wf16 = np.ascontiguousarray(Wf.astype(np.float16))
    wa16 = np.ascontiguousarray(Wa.astype(np.float16))
    wft16 = np.ascontiguousarray(Wf.T.astype(np.float16))
    wat16 = np.ascontiguousarray(Wa.T.astype(np.float16))
    bfb = np.ascontiguousarray(np.tile(bf.astype(np.float16), (P, GRP // P)))
    bzb = np.ascontiguousarray(np.tile((bf + ba).astype(np.float16), (P, 1)))
    lbf = np.ascontiguousarray((float(L) * bf).astype(np.float16)[None, :])
    bar = np.ascontiguousarray(ba.astype(np.float16)[None, :])

    in_maps = []
    for c in range(NCORES):
        sl = slice(c * BPC, (c + 1) * BPC)
        in_maps.append(
            {
                "x": np.ascontiguousarray(x16[sl]),
                "y": np.ascontiguousarray(y16[sl]),
                "wf": wf16,
                "wa": wa16,
                "wft": wft16,
                "wat": wat16,
                "bfb": bfb,
                "bzb": bzb,
                "lbf": lbf,
                "bar": bar,
            }
        )
    return in_maps


def kernel(x, y, Wf, bf, Wa, ba):
    from concourse.bass_utils import run_bass_kernel_spmd

    nc = _get_nc()
    in_maps = make_in_maps(x, y, Wf, bf, Wa, ba)
    res = run_bass_kernel_spmd(nc, in_maps, core_ids=list(range(NCORES)))
    out = np.concatenate([r["out"] for r in res.results], axis=0)
    return np.ascontiguousarray(out.astype(np.float32))


if __name__ == "__main__":
    rng = np.random.default_rng(0)
    inputs = {
        "x": rng.standard_normal((B, P, L), dtype=np.float32),
        "y": rng.standard_normal((B, P, L), dtype=np.float32),
        "Wf": (rng.standard_normal((P, P)) / np.sqrt(P)).astype(np.float32),
        "bf": (rng.standard_normal(P) * 0.02).astype(np.float32),
        "Wa": (rng.standard_normal((P, P)) / np.sqrt(P)).astype(np.float32),
        "ba": (rng.standard_normal(P) * 0.02).astype(np.float32),
    }
    o = kernel(**inputs)
    print(o.shape, o.dtype)
